# revision 12
# baseline (speedup 1.0000x reference)
"""DRL4TSP pointer-network decoder on 8 Trainium2 NeuronCores (Bass/Tile).

Pure data parallel: B=1024 -> 128 per core; weights replicated; each core runs
the full 128-step sequential decode on its batch slice.

Restructured math (validated bit-stable vs the jax reference in fp32 numpy:
0/131072 pointer mismatches, logp rel err 1.9e-6):
  - static/dynamic encoders are rank<=3 in (h,n):  E1 = A1 @ feat[b] with
    A1 = [W1@enc_s_w | W2@enc_d_w]  (K=3 matmul);  E2 = A2 @ coords (K=2).
  - context = (attn-weighted coords) @ enc_s_w.T  (N-reduction in 2-dim
    coordinate space instead of H x N).
  - sigmoid(x) = 0.5*tanh(0.5x)+0.5 keeps the whole loop on the single
    `exp_and_others` ACT table set (tanh+exp) -> no table swaps.
  - softmax without max-shift for the attention (values are tiny); logp
    accumulated as -log(sum_exp) per step, log taken once at the end.
  - argmax via is_equal + descending-iota max (first-index tie-break),
    one-hot gather for last_out (exact).
All matmuls fp32: fp32r (11-bit mantissa) / bf16 would flip decode
trajectories (min top-2 logit gap in the reference is ~1.3e-6).

Engine mapping per step: PE does the K=3/K=2 arg matmuls (4x row-group
packed), the v-reductions (4x col-group packed, M=1 at partitions
{0,32,64,96}), GRU/small matmuls and transposes; ACT does tanh/exp; DVE does
the c1 broadcast-add into PSUM, softmax reductions, argmax trick and PSUM
evacuations; al rows return to [b, n] layout via SBUF->SBUF DMA gathers.

Built as bacc.Bacc and nc.compile()d: the generate_event_semaphores pass
splits multi-semaphore waits (TRN2 instructions accept only one sync wait).
"""
import sys

if "/opt/trn_rl_repo" not in sys.path:
    sys.path.insert(0, "/opt/trn_rl_repo")

import numpy as np
from contextlib import ExitStack

import concourse.bass as bass
import concourse.mybir as mybir
import concourse.tile as tile
from concourse import bacc
from concourse.bass_utils import run_bass_kernel_spmd

f32 = mybir.dt.float32
i32 = mybir.dt.int32
AF = mybir.ActivationFunctionType
OP = mybir.AluOpType
AX = mybir.AxisListType

B, S, D, H, N = 1024, 2, 1, 128, 128
NCORES = 8
BL = B // NCORES            # 128 local batch
CHUNK = 512
NCHUNK = (BL * N) // CHUNK  # 32 chunks per attention block
PHASE_CH = 2                # chunks per arg-psum phase (1024 cols)
GEN_CH = 4                  # chunks per al-psum generation
GATH_CH = 8                 # chunks per DMA gather (2 gens)

_cache = {}


def _bcast_n(ap2d, b0, nb, nn=N):
    sl = ap2d[:, b0:b0 + nb]
    return bass.AP(tensor=sl.tensor, offset=sl.offset,
                   ap=[sl.ap[0], sl.ap[1], [0, nn]])


def _sbcast(ap2d, ns):
    return bass.AP(tensor=ap2d.tensor, offset=ap2d.offset,
                   ap=[ap2d.ap[0], [0, ns], ap2d.ap[1]])


# ---------------- const blob layout (shared by host prep and program) --------
_SEGS = [
    ("feat_stack", 128, BL * N),
    ("static_bsn", BL, S * N),
    ("A1_stack", 128, H),
    ("A2_stack", 128, H),
    ("ident", H, H),
    ("W3T", H, H),
    ("V2T", H, H),
    ("wihT", H, 3 * H),
    ("whhT", H, 3 * H),
    ("embT_w", S, H),
    ("encswT", S, H),
    ("v_col", H, 1),
    ("dv_col", H, 1),
    ("bias1_row", 1, H),
    ("bias2_row", 1, H),
    ("embb_row", 1, H),
    ("encsb_row", 1, H),
    ("ones_row", 1, BL),
    ("gbias_rep", 128, 4 * H),
    ("x0T_rep", S, BL),
    ("iota_rep", 128, N),
    ("iota_desc", 128, N),
]
_SEG_OFF = {}
_BLOB_COLS = 0
for _nm, _r, _c in _SEGS:
    _SEG_OFF[_nm] = (_BLOB_COLS, _r, _c)
    _BLOB_COLS += _c


def _build_program():
    nc = bacc.Bacc("TRN2", target_bir_lowering=False, debug=False,
                   num_devices=NCORES)
    blob_d = nc.dram_tensor("blob", [128, _BLOB_COLS], f32,
                            kind="ExternalInput").ap()
    out_idx = nc.dram_tensor("out_idx", [BL, N], i32, kind="ExternalOutput").ap()
    out_logp = nc.dram_tensor("out_logp", [BL, 1], f32, kind="ExternalOutput").ap()
    dbg_al = nc.dram_tensor("dbg_al", [BL, N], f32, kind="ExternalOutput").ap()
    dbg_lg = nc.dram_tensor("dbg_lg", [BL, N], f32, kind="ExternalOutput").ap()
    dbg_c1 = nc.dram_tensor("dbg_c1", [H, BL], f32, kind="ExternalOutput").ap()
    dbg_h = nc.dram_tensor("dbg_h", [BL, H], f32, kind="ExternalOutput").ap()

    with tile.TileContext(nc) as tc, ExitStack() as ctx:
        const = ctx.enter_context(tc.tile_pool(name="const", bufs=1))
        state = ctx.enter_context(tc.tile_pool(name="state", bufs=1))
        upool = ctx.enter_context(tc.tile_pool(name="upool", bufs=3))
        small = ctx.enter_context(tc.tile_pool(name="small", bufs=4))
        alsb = ctx.enter_context(tc.tile_pool(name="alsb", bufs=2))
        stgp = ctx.enter_context(tc.tile_pool(name="stgp", bufs=2))
        argp = ctx.enter_context(tc.tile_pool(name="argp", bufs=2, space="PSUM"))
        alp = ctx.enter_context(tc.tile_pool(name="alp", bufs=2, space="PSUM"))
        mscp = ctx.enter_context(tc.tile_pool(name="mscp", bufs=1, space="PSUM"))

        blob = const.tile([128, _BLOB_COLS], f32)
        nc.sync.dma_start(blob[:], blob_d[:])

        def cv(name):
            c0, rows, w = _SEG_OFF[name]
            return blob[:rows, c0:c0 + w]

        feat = cv("feat_stack")
        static_bsn = cv("static_bsn")
        ident = cv("ident")
        ones_row = cv("ones_row")

        # persistent state
        hT = state.tile([H, BL], f32)
        h_b = state.tile([BL, H], f32)
        lastT = state.tile([S, BL], f32)
        ptrs_f = state.tile([BL, N], f32)
        s2s = state.tile([BL, N], f32)
        c1T = state.tile([H, BL], f32)
        c2T = state.tile([H, BL], f32)
        nc.vector.tensor_copy(lastT[:], cv("x0T_rep"))
        nc.vector.memset(hT[:], 0.0)
        nc.vector.memset(h_b[:], 0.0)

        mm = nc.tensor.matmul

        def small_mm(out_sl, pairs, msc, n0):
            p = out_sl.shape[0]
            w = out_sl.shape[1]
            for i, (lhsT, rhs) in enumerate(pairs):
                mm(msc[:p, n0:n0 + w], lhsT, rhs,
                   start=(i == 0), stop=(i == len(pairs) - 1))
            return nc.vector.tensor_copy(out_sl, msc[:p, n0:n0 + w])

        def attention_block(A_stack, kdim, cT, red_col, al_l):
            """arg = A@feat + cT bcast; u = tanh(arg); al = red_col.T @ u;
            al gathered back to [b, n] layout into al_l."""
            al_sb = alsb.tile([BL, N], f32, tag="al_raw")
            stage = None
            alpsA = alpsB = None
            nph = NCHUNK // PHASE_CH  # 16
            for p in range(nph):
                arg = argp.tile([128, PHASE_CH * CHUNK], f32, tag="arg")
                for ci in range(PHASE_CH):
                    c = p * PHASE_CH + ci
                    g = c % 4
                    mm(arg[:, bass.ts(ci, CHUNK)],
                       A_stack[32 * g:32 * g + kdim, :],
                       feat[32 * g:32 * g + kdim, bass.ts(c, CHUNK)],
                       start=True, stop=True, tile_position=(32 * g, 0))
                    nc.vector.tensor_tensor(arg[:, bass.ts(ci, CHUNK)],
                                            arg[:, bass.ts(ci, CHUNK)],
                                            _bcast_n(cT, 4 * c, 4), op=OP.add)
                u = upool.tile([128, PHASE_CH * CHUNK], f32, tag="u")
                nc.scalar.activation(u[:], arg[:], AF.Tanh)
                for ci in range(PHASE_CH):
                    c = p * PHASE_CH + ci
                    cl = c % GATH_CH        # position within gather group
                    # gather stream iterates (colgroup, slot, b_in, n) with
                    # colgroup outermost -> chunk cl maps to colgroup cl//2,
                    # stage slot cl%2 so dst rows come out b-ascending.
                    if cl == 0:
                        alpsA = alp.tile([128, CHUNK], f32, tag="al")
                        stage = stgp.tile([128, GATH_CH * CHUNK // 4], f32,
                                          tag="stage")
                    if cl == 1:
                        alpsB = alp.tile([128, CHUNK], f32, tag="al")
                    tsel = alpsA if cl % 2 == 0 else alpsB
                    gc = cl // 2
                    mm(tsel[32 * gc:32 * gc + 1, :], red_col,
                       u[:, bass.ts(ci, CHUNK)],
                       start=True, stop=True, tile_position=(0, 32 * gc))
                    if cl == GATH_CH - 2:
                        nc.vector.tensor_copy(stage[:, 0:CHUNK], alpsA[:])
                    if cl == GATH_CH - 1:
                        nc.vector.tensor_copy(stage[:, CHUNK:2 * CHUNK], alpsB[:])
                        q = c // GATH_CH
                        nc.sync.dma_start(al_sb[32 * q:32 * q + 32, :],
                                          stage[::32, :])
                        nc.vector.tensor_copy(al_l[32 * q:32 * q + 32, :],
                                              al_sb[32 * q:32 * q + 32, :])

        for t in range(N):
            msc = mscp.tile([128, 512], f32, tag="msc")
            embT = small.tile([H, BL], f32, tag="embT")
            small_mm(embT[:], [(cv("embT_w"), lastT[:]),
                               (cv("embb_row"), ones_row)], msc, 0)
            gg_ps = mscp.tile([128, 512], f32, tag="gg")
            mm(gg_ps[:, 0:256], embT[:], cv("wihT")[:, 0:256], start=True, stop=False)
            mm(gg_ps[:, 0:256], hT[:], cv("whhT")[:, 0:256], start=False, stop=True)
            mm(gg_ps[:, 256:384], embT[:], cv("wihT")[:, 256:384], start=True, stop=True)
            mm(gg_ps[:, 384:512], hT[:], cv("whhT")[:, 256:384], start=True, stop=True)
            gg = small.tile([BL, 512], f32, tag="gg_sb")
            nc.vector.tensor_tensor(gg[:], gg_ps[:], cv("gbias_rep"), op=OP.add)
            rz_t = small.tile([BL, 256], f32, tag="rz_t")
            nc.scalar.activation(rz_t[:], gg[:, 0:256], AF.Tanh, scale=0.5)
            rz = small.tile([BL, 256], f32, tag="rz")
            nc.vector.tensor_scalar(rz[:], rz_t[:], 0.5, 0.5, op0=OP.mult, op1=OP.add)
            rh = small.tile([BL, H], f32, tag="rh")
            nc.vector.tensor_tensor(rh[:], rz[:, 0:128], gg[:, 384:512], op=OP.mult)
            argn = small.tile([BL, H], f32, tag="argn")
            nc.vector.tensor_tensor(argn[:], rh[:], gg[:, 256:384], op=OP.add)
            ng = small.tile([BL, H], f32, tag="ng")
            nc.scalar.activation(ng[:], argn[:], AF.Tanh)
            hd = small.tile([BL, H], f32, tag="hd")
            nc.vector.tensor_tensor(hd[:], h_b[:], ng[:], op=OP.subtract)
            zd = small.tile([BL, H], f32, tag="zd")
            nc.vector.tensor_tensor(zd[:], rz[:, 128:256], hd[:], op=OP.mult)
            nc.vector.tensor_tensor(h_b[:], ng[:], zd[:], op=OP.add)
            mm(msc[:, 256:384], h_b[:], ident, is_transpose=True)
            nc.vector.tensor_copy(hT[:], msc[:, 256:384])
            small_mm(c1T[:], [(cv("W3T"), hT[:]),
                              (cv("bias1_row"), ones_row)], msc, 384)

            al_l = alsb.tile([BL, N], f32, tag="al_l")
            attention_block(cv("A1_stack"), 3, c1T, cv("v_col"), al_l)
            if t == 0:
                nc.sync.dma_start(dbg_al[:], al_l[:])
                nc.sync.dma_start(dbg_c1[:], c1T[:])
                nc.sync.dma_start(dbg_h[:], h_b[:])

            e1 = small.tile([BL, N], f32, tag="e1")
            s1 = small.tile([BL, 1], f32, tag="s1")
            nc.scalar.activation(e1[:], al_l[:], AF.Exp, accum_out=s1[:])
            rs1 = small.tile([BL, 1], f32, tag="rs1")
            nc.vector.reciprocal(rs1[:], s1[:])
            prod = small.tile([BL, S * N], f32, tag="prod")
            nc.vector.tensor_tensor(prod[:], _sbcast(e1[:], S), static_bsn,
                                    op=OP.mult)
            cs_u = small.tile([BL, S], f32, tag="cs_u")
            nc.vector.tensor_reduce(cs_u[:], prod[:].rearrange("p (s n) -> p s n", s=S),
                                    axis=AX.X, op=OP.add)
            cs = small.tile([BL, S], f32, tag="cs")
            nc.vector.tensor_scalar(cs[:], cs_u[:], rs1[:], None, op0=OP.mult)
            msc2 = mscp.tile([128, 512], f32, tag="msc")
            mm(msc2[:S, 0:BL], cs[:], ident, is_transpose=True)
            csT = small.tile([S, BL], f32, tag="csT")
            nc.vector.tensor_copy(csT[:], msc2[:S, 0:BL])
            ctxT = small.tile([H, BL], f32, tag="ctxT")
            small_mm(ctxT[:], [(cv("encswT"), csT[:]),
                               (cv("encsb_row"), ones_row)], msc2, 128)
            small_mm(c2T[:], [(cv("V2T"), ctxT[:]),
                              (cv("bias2_row"), ones_row)], msc2, 256)

            lg_l = alsb.tile([BL, N], f32, tag="lg_l")
            attention_block(cv("A2_stack"), 2, c2T, cv("dv_col"), lg_l)
            if t == 0:
                nc.sync.dma_start(dbg_lg[:], lg_l[:])

            m2 = small.tile([BL, 1], f32, tag="m2")
            nc.vector.tensor_reduce(m2[:], lg_l[:], axis=AX.X, op=OP.max)
            nm2 = small.tile([BL, 1], f32, tag="nm2")
            nc.vector.tensor_scalar(nm2[:], m2[:], -1.0, None, op0=OP.mult)
            e2 = small.tile([BL, N], f32, tag="e2")
            nc.scalar.activation(e2[:], lg_l[:], AF.Exp, bias=nm2[:],
                                 accum_out=s2s[:, t:t + 1])
            mask = small.tile([BL, N], f32, tag="mask")
            nc.vector.tensor_scalar(mask[:], lg_l[:], m2[:], None, op0=OP.is_equal)
            idxv = small.tile([BL, N], f32, tag="idxv")
            nc.vector.tensor_tensor(idxv[:], mask[:], cv("iota_desc"), op=OP.mult)
            rmax = small.tile([BL, 1], f32, tag="rmax")
            nc.vector.tensor_reduce(rmax[:], idxv[:], axis=AX.X, op=OP.max)
            nc.vector.tensor_scalar(ptrs_f[:, t:t + 1], rmax[:], -1.0, float(N - 1),
                                    op0=OP.mult, op1=OP.add)
            oh = small.tile([BL, N], f32, tag="oh")
            nc.vector.tensor_scalar(oh[:], cv("iota_rep"), ptrs_f[:, t:t + 1],
                                    None, op0=OP.is_equal)
            lprod = small.tile([BL, S * N], f32, tag="lprod")
            nc.vector.tensor_tensor(lprod[:], _sbcast(oh[:], S), static_bsn,
                                    op=OP.mult)
            last_b = small.tile([BL, S], f32, tag="last_b")
            nc.vector.tensor_reduce(last_b[:], lprod[:].rearrange("p (s n) -> p s n", s=S),
                                    axis=AX.X, op=OP.add)
            msc3 = mscp.tile([128, 512], f32, tag="msc")
            mm(msc3[:S, 128:128 + BL], last_b[:], ident, is_transpose=True)
            nc.vector.tensor_copy(lastT[:], msc3[:S, 128:128 + BL])

        ptr_i = state.tile([BL, N], i32)
        nc.vector.tensor_copy(ptr_i[:], ptrs_f[:])
        nc.sync.dma_start(out_idx[:], ptr_i[:])
        lg = state.tile([BL, N], f32)
        nc.scalar.activation(lg[:], s2s[:], AF.Ln)
        lsum = state.tile([BL, 1], f32)
        nc.vector.tensor_reduce(lsum[:], lg[:], axis=AX.X, op=OP.add)
        logp = state.tile([BL, 1], f32)
        nc.vector.tensor_scalar(logp[:], lsum[:], -1.0, None, op0=OP.mult)
        nc.sync.dma_start(out_logp[:], logp[:])

    nc.compile()
    return nc


def _host_prep(inputs):
    f = np.float32
    g = {k: np.asarray(v) for k, v in inputs.items()}
    attn_W, dec_W = g["attn_W"].astype(np.float64), g["dec_W"].astype(np.float64)
    enc_s_w, enc_d_w = g["enc_s_w"].astype(np.float64), g["enc_d_w"].astype(np.float64)
    W1, W2 = attn_W[:, :H], attn_W[:, H:2 * H]
    V1 = dec_W[:, :H]
    A1 = np.concatenate([W1 @ enc_s_w, W2 @ enc_d_w], axis=1).astype(f)
    A2 = (V1 @ enc_s_w).astype(f)
    bias1 = (W1 @ g["enc_s_b"].astype(np.float64)
             + W2 @ g["enc_d_b"].astype(np.float64)).astype(f)
    bias2 = (V1 @ g["enc_s_b"].astype(np.float64)).astype(f)

    A1_stack = np.zeros((128, H), f)
    A2_stack = np.zeros((128, H), f)
    for q in range(4):
        A1_stack[32 * q:32 * q + 3, :] = A1.T
        A2_stack[32 * q:32 * q + 2, :] = A2.T

    gbias = np.zeros((4 * H,), f)
    gbias[0:2 * H] = (g["b_ih"][0:2 * H] + g["b_hh"][0:2 * H]).astype(f)
    gbias[2 * H:3 * H] = g["b_ih"][2 * H:3 * H].astype(f)
    gbias[3 * H:4 * H] = g["b_hh"][2 * H:3 * H].astype(f)

    iota = np.arange(N, dtype=f)
    shared = {
        "A1_stack": A1_stack, "A2_stack": A2_stack,
        "ident": np.eye(H, dtype=f),
        "W3T": np.ascontiguousarray(g["attn_W"][:, 2 * H:].T.astype(f)),
        "V2T": np.ascontiguousarray(g["dec_W"][:, H:].T.astype(f)),
        "wihT": np.ascontiguousarray(g["w_ih"].T.astype(f)),
        "whhT": np.ascontiguousarray(g["w_hh"].T.astype(f)),
        "embT_w": np.ascontiguousarray(g["emb_w"].T.astype(f)),
        "encswT": np.ascontiguousarray(g["enc_s_w"].T.astype(f)),
        "v_col": np.ascontiguousarray(g["attn_v"].T.astype(f)),
        "dv_col": np.ascontiguousarray(g["dec_v"].T.astype(f)),
        "bias1_row": bias1[None, :], "bias2_row": bias2[None, :],
        "embb_row": g["emb_b"].astype(f)[None, :],
        "encsb_row": g["enc_s_b"].astype(f)[None, :],
        "ones_row": np.ones((1, BL), f),
        "gbias_rep": np.broadcast_to(gbias, (128, 4 * H)),
        "x0T_rep": np.broadcast_to(g["x0"].astype(f).T, (S, BL)),
        "iota_rep": np.broadcast_to(iota, (128, N)),
        "iota_desc": np.broadcast_to((N - 1) - iota, (128, N)),
    }

    static = g["static"].astype(f)
    dynamic = g["dynamic"].astype(f)
    in_maps = []
    for c in range(NCORES):
        sl = slice(c * BL, (c + 1) * BL)
        st, dy = static[sl], dynamic[sl]
        featb = np.zeros((128, BL * N), f)
        for q in range(4):
            featb[32 * q + 0, :] = st[:, 0, :].reshape(-1)
            featb[32 * q + 1, :] = st[:, 1, :].reshape(-1)
            featb[32 * q + 2, :] = dy[:, 0, :].reshape(-1)
        per = dict(shared)
        per["feat_stack"] = featb
        per["static_bsn"] = st.reshape(BL, S * N)
        blobarr = np.zeros((128, _BLOB_COLS), f)
        for nm, (c0, rows, w) in _SEG_OFF.items():
            blobarr[:rows, c0:c0 + w] = per[nm]
        in_maps.append({"blob": blobarr})
    return in_maps


def kernel(**inputs):
    if "nc" not in _cache:
        _cache["nc"] = _build_program()
    in_maps = _host_prep(inputs)
    res = run_bass_kernel_spmd(_cache["nc"], in_maps, list(range(NCORES)))
    idx = np.concatenate([r["out_idx"] for r in res.results], axis=0)
    logp = np.concatenate([r["out_logp"][:, 0] for r in res.results], axis=0)
    return idx.astype(np.int32), logp.astype(np.float32)


# revision 13
# speedup vs baseline: 1.6032x; 1.6032x over previous
"""DRL4TSP pointer-network decoder on 8 Trainium2 NeuronCores (Bass/Tile).

Pure data parallel: B=1024 -> 128 per core; weights replicated; each core runs
the full 128-step sequential decode on its batch slice.

Restructured math (validated bit-stable vs the jax reference in fp32 numpy:
0/131072 pointer mismatches, logp rel err 1.9e-6):
  - static/dynamic encoders are rank<=3 in (h,n):  E1 = A1 @ feat[b] with
    A1 = [W1@enc_s_w | W2@enc_d_w]  (K=3 matmul);  E2 = A2 @ coords (K=2).
  - context = (attn-weighted coords) @ enc_s_w.T  (N-reduction in 2-dim
    coordinate space instead of H x N).
  - sigmoid(x) = 0.5*tanh(0.5x)+0.5 keeps the whole loop on the single
    `exp_and_others` ACT table set (tanh+exp) -> no table swaps.
  - softmax without max-shift for the attention (values are tiny); logp
    accumulated as -log(sum_exp) per step, log taken once at the end.
  - argmax via is_equal + descending-iota max (first-index tie-break),
    one-hot gather for last_out (exact).
All matmuls fp32: fp32r (11-bit mantissa) / bf16 would flip decode
trajectories (min top-2 logit gap in the reference is ~1.3e-6).

Engine mapping per step: PE does the K=3/K=2 arg matmuls (4x row-group
packed), the v-reductions (4x col-group packed, M=1 at partitions
{0,32,64,96}), GRU/small matmuls and transposes; ACT does tanh/exp; DVE does
the c1 broadcast-add into PSUM, softmax reductions, argmax trick and PSUM
evacuations; al rows return to [b, n] layout via SBUF->SBUF DMA gathers.

Built as bacc.Bacc and nc.compile()d: the generate_event_semaphores pass
splits multi-semaphore waits (TRN2 instructions accept only one sync wait).
"""
import sys

if "/opt/trn_rl_repo" not in sys.path:
    sys.path.insert(0, "/opt/trn_rl_repo")

import numpy as np
from contextlib import ExitStack

import concourse.bass as bass
import concourse.mybir as mybir
import concourse.tile as tile
from concourse import bacc
from concourse.bass_utils import run_bass_kernel_spmd

f32 = mybir.dt.float32
i32 = mybir.dt.int32
AF = mybir.ActivationFunctionType
OP = mybir.AluOpType
AX = mybir.AxisListType

B, S, D, H, N = 1024, 2, 1, 128, 128
NCORES = 8
BL = B // NCORES            # 128 local batch
CHUNK = 512
NCHUNK = (BL * N) // CHUNK  # 32 chunks per attention block
PHASE_CH = 2                # chunks per arg-psum phase (1024 cols)
GEN_CH = 4                  # chunks per al-psum generation
GATH_CH = 8                 # chunks per DMA gather (2 gens)

_cache = {}


def _bcast_n(ap2d, b0, nb, nn=N):
    sl = ap2d[:, b0:b0 + nb]
    return bass.AP(tensor=sl.tensor, offset=sl.offset,
                   ap=[sl.ap[0], sl.ap[1], [0, nn]])


def _sbcast(ap2d, ns):
    return bass.AP(tensor=ap2d.tensor, offset=ap2d.offset,
                   ap=[ap2d.ap[0], [0, ns], ap2d.ap[1]])


# ---------------- const blob layout (shared by host prep and program) --------
_SEGS = [
    ("static_bsn", BL, S * N),
    ("A1_stack", 128, H),
    ("A2_stack", 128, H),
    ("ident", H, H),
    ("W3T", H, H),
    ("V2T", H, H),
    ("wihT", H, 3 * H),
    ("whhT", H, 3 * H),
    ("embT_w", S, H),
    ("encswT", S, H),
    ("v_col", H, 1),
    ("dv_col", H, 1),
    ("bias1_row", 1, H),
    ("bias2_row", 1, H),
    ("embb_row", 1, H),
    ("encsb_row", 1, H),
    ("ones_row", 1, BL),
    ("gbias_rep", 128, 4 * H),
    ("x0T_rep", S, BL),
    ("iota_rep", 128, N),
    ("iota_desc", 128, N),
]
_SEG_OFF = {}
_BLOB_COLS = 0
for _nm, _r, _c in _SEGS:
    _SEG_OFF[_nm] = (_BLOB_COLS, _r, _c)
    _BLOB_COLS += _c


def _build_program():
    nc = bacc.Bacc("TRN2", target_bir_lowering=False, debug=False,
                   num_devices=NCORES)
    blob_d = nc.dram_tensor("blob", [128, _BLOB_COLS], f32,
                            kind="ExternalInput").ap()
    featd = nc.dram_tensor("featd", [3, BL * N], f32,
                           kind="ExternalInput").ap()
    out_idx = nc.dram_tensor("out_idx", [BL, N], i32, kind="ExternalOutput").ap()
    out_logp = nc.dram_tensor("out_logp", [BL, 1], f32, kind="ExternalOutput").ap()

    with tile.TileContext(nc) as tc, ExitStack() as ctx:
        const = ctx.enter_context(tc.tile_pool(name="const", bufs=1))
        state = ctx.enter_context(tc.tile_pool(name="state", bufs=1))
        upool = ctx.enter_context(tc.tile_pool(name="upool", bufs=3))
        small = ctx.enter_context(tc.tile_pool(name="small", bufs=4))
        alsb = ctx.enter_context(tc.tile_pool(name="alsb", bufs=2))
        stgp = ctx.enter_context(tc.tile_pool(name="stgp", bufs=2))
        argp = ctx.enter_context(tc.tile_pool(name="argp", bufs=2, space="PSUM"))
        alp = ctx.enter_context(tc.tile_pool(name="alp", bufs=2, space="PSUM"))
        mscp = ctx.enter_context(tc.tile_pool(name="mscp", bufs=1, space="PSUM"))

        blob = const.tile([128, _BLOB_COLS], f32)
        nc.sync.dma_start(blob[:], blob_d[:])
        feat_stack = const.tile([128, BL * N], f32)
        for q4 in range(4):
            nc.sync.dma_start(feat_stack[32 * q4:32 * q4 + 3, :], featd[:])

        def cv(name):
            c0, rows, w = _SEG_OFF[name]
            return blob[:rows, c0:c0 + w]

        feat = feat_stack[:]
        static_bsn = cv("static_bsn")
        ident = cv("ident")
        ones_row = cv("ones_row")

        # persistent state
        hT = state.tile([H, BL], f32)
        h_b = state.tile([BL, H], f32)
        lastT = state.tile([S, BL], f32)
        ptrs_f = state.tile([BL, N], f32)
        s2s = state.tile([BL, N], f32)
        c1T = state.tile([H, BL], f32)
        c2T = state.tile([H, BL], f32)
        nc.vector.tensor_copy(lastT[:], cv("x0T_rep"))
        nc.vector.memset(hT[:], 0.0)
        nc.vector.memset(h_b[:], 0.0)

        mm = nc.tensor.matmul

        def small_mm(out_sl, pairs, msc, n0):
            p = out_sl.shape[0]
            w = out_sl.shape[1]
            for i, (lhsT, rhs) in enumerate(pairs):
                mm(msc[:p, n0:n0 + w], lhsT, rhs,
                   start=(i == 0), stop=(i == len(pairs) - 1))
            return nc.vector.tensor_copy(out_sl, msc[:p, n0:n0 + w])

        def attention_block(A_stack, kdim, cT, red_col, al_l):
            """arg = A@feat + cT bcast; u = tanh(arg); al = red_col.T @ u;
            al gathered back to [b, n] layout into al_l."""
            al_sb = alsb.tile([BL, N], f32, tag="al_raw")
            stage = None
            alpsA = alpsB = None
            nph = NCHUNK // PHASE_CH  # 16
            for p in range(nph):
                arg = argp.tile([128, PHASE_CH * CHUNK], f32, tag="arg")
                for ci in range(PHASE_CH):
                    c = p * PHASE_CH + ci
                    g = c % 4
                    mm(arg[:, bass.ts(ci, CHUNK)],
                       A_stack[32 * g:32 * g + kdim, :],
                       feat[32 * g:32 * g + kdim, bass.ts(c, CHUNK)],
                       start=True, stop=True, tile_position=(32 * g, 0))
                nc.vector.tensor_tensor(arg[:], arg[:],
                                        _bcast_n(cT, 4 * p * PHASE_CH,
                                                 4 * PHASE_CH), op=OP.add)
                u = upool.tile([128, PHASE_CH * CHUNK], f32, tag="u")
                nc.scalar.activation(u[:], arg[:], AF.Tanh)
                for ci in range(PHASE_CH):
                    c = p * PHASE_CH + ci
                    cl = c % GATH_CH        # position within gather group
                    # gather stream iterates (colgroup, slot, b_in, n) with
                    # colgroup outermost -> chunk cl maps to colgroup cl//2,
                    # stage slot cl%2 so dst rows come out b-ascending.
                    if cl == 0:
                        alpsA = alp.tile([128, CHUNK], f32, tag="al")
                        stage = stgp.tile([128, GATH_CH * CHUNK // 4], f32,
                                          tag="stage")
                    if cl == 1:
                        alpsB = alp.tile([128, CHUNK], f32, tag="al")
                    tsel = alpsA if cl % 2 == 0 else alpsB
                    gc = cl // 2
                    mm(tsel[32 * gc:32 * gc + 1, :], red_col,
                       u[:, bass.ts(ci, CHUNK)],
                       start=True, stop=True, tile_position=(0, 32 * gc))
                    if cl == GATH_CH - 2:
                        nc.vector.tensor_copy(stage[:, 0:CHUNK], alpsA[:])
                    if cl == GATH_CH - 1:
                        nc.vector.tensor_copy(stage[:, CHUNK:2 * CHUNK], alpsB[:])
                        q = c // GATH_CH
                        nc.sync.dma_start(al_sb[32 * q:32 * q + 32, :],
                                          stage[::32, :])
                        nc.vector.tensor_copy(al_l[32 * q:32 * q + 32, :],
                                              al_sb[32 * q:32 * q + 32, :])

        for t in range(N):
            msc = mscp.tile([128, 512], f32, tag="msc")
            embT = small.tile([H, BL], f32, tag="embT")
            small_mm(embT[:], [(cv("embT_w"), lastT[:]),
                               (cv("embb_row"), ones_row)], msc, 0)
            gg_ps = mscp.tile([128, 512], f32, tag="gg")
            mm(gg_ps[:, 0:256], embT[:], cv("wihT")[:, 0:256], start=True, stop=False)
            mm(gg_ps[:, 0:256], hT[:], cv("whhT")[:, 0:256], start=False, stop=True)
            mm(gg_ps[:, 256:384], embT[:], cv("wihT")[:, 256:384], start=True, stop=True)
            mm(gg_ps[:, 384:512], hT[:], cv("whhT")[:, 256:384], start=True, stop=True)
            gg = small.tile([BL, 512], f32, tag="gg_sb")
            nc.vector.tensor_tensor(gg[:], gg_ps[:], cv("gbias_rep"), op=OP.add)
            rz_t = small.tile([BL, 256], f32, tag="rz_t")
            nc.scalar.activation(rz_t[:], gg[:, 0:256], AF.Tanh, scale=0.5)
            rz = small.tile([BL, 256], f32, tag="rz")
            nc.vector.tensor_scalar(rz[:], rz_t[:], 0.5, 0.5, op0=OP.mult, op1=OP.add)
            rh = small.tile([BL, H], f32, tag="rh")
            nc.vector.tensor_tensor(rh[:], rz[:, 0:128], gg[:, 384:512], op=OP.mult)
            argn = small.tile([BL, H], f32, tag="argn")
            nc.vector.tensor_tensor(argn[:], rh[:], gg[:, 256:384], op=OP.add)
            ng = small.tile([BL, H], f32, tag="ng")
            nc.scalar.activation(ng[:], argn[:], AF.Tanh)
            hd = small.tile([BL, H], f32, tag="hd")
            nc.vector.tensor_tensor(hd[:], h_b[:], ng[:], op=OP.subtract)
            zd = small.tile([BL, H], f32, tag="zd")
            nc.vector.tensor_tensor(zd[:], rz[:, 128:256], hd[:], op=OP.mult)
            nc.vector.tensor_tensor(h_b[:], ng[:], zd[:], op=OP.add)
            mm(msc[:, 256:384], h_b[:], ident, is_transpose=True)
            nc.vector.tensor_copy(hT[:], msc[:, 256:384])
            small_mm(c1T[:], [(cv("W3T"), hT[:]),
                              (cv("bias1_row"), ones_row)], msc, 384)

            al_l = alsb.tile([BL, N], f32, tag="al_l")
            attention_block(cv("A1_stack"), 3, c1T, cv("v_col"), al_l)

            e1 = small.tile([BL, N], f32, tag="e1")
            s1 = small.tile([BL, 1], f32, tag="s1")
            nc.scalar.activation(e1[:], al_l[:], AF.Exp, accum_out=s1[:])
            rs1 = small.tile([BL, 1], f32, tag="rs1")
            nc.vector.reciprocal(rs1[:], s1[:])
            prod = small.tile([BL, S * N], f32, tag="prod")
            nc.vector.tensor_tensor(prod[:], _sbcast(e1[:], S), static_bsn,
                                    op=OP.mult)
            cs_u = small.tile([BL, S], f32, tag="cs_u")
            nc.vector.tensor_reduce(cs_u[:], prod[:].rearrange("p (s n) -> p s n", s=S),
                                    axis=AX.X, op=OP.add)
            cs = small.tile([BL, S], f32, tag="cs")
            nc.vector.tensor_scalar(cs[:], cs_u[:], rs1[:], None, op0=OP.mult)
            msc2 = mscp.tile([128, 512], f32, tag="msc")
            mm(msc2[:S, 0:BL], cs[:], ident, is_transpose=True)
            csT = small.tile([S, BL], f32, tag="csT")
            nc.vector.tensor_copy(csT[:], msc2[:S, 0:BL])
            ctxT = small.tile([H, BL], f32, tag="ctxT")
            small_mm(ctxT[:], [(cv("encswT"), csT[:]),
                               (cv("encsb_row"), ones_row)], msc2, 128)
            small_mm(c2T[:], [(cv("V2T"), ctxT[:]),
                              (cv("bias2_row"), ones_row)], msc2, 256)

            lg_l = alsb.tile([BL, N], f32, tag="lg_l")
            attention_block(cv("A2_stack"), 2, c2T, cv("dv_col"), lg_l)

            m2 = small.tile([BL, 1], f32, tag="m2")
            nc.vector.tensor_reduce(m2[:], lg_l[:], axis=AX.X, op=OP.max)
            nm2 = small.tile([BL, 1], f32, tag="nm2")
            nc.vector.tensor_scalar(nm2[:], m2[:], -1.0, None, op0=OP.mult)
            e2 = small.tile([BL, N], f32, tag="e2")
            nc.scalar.activation(e2[:], lg_l[:], AF.Exp, bias=nm2[:],
                                 accum_out=s2s[:, t:t + 1])
            mask = small.tile([BL, N], f32, tag="mask")
            nc.vector.tensor_scalar(mask[:], lg_l[:], m2[:], None, op0=OP.is_equal)
            idxv = small.tile([BL, N], f32, tag="idxv")
            nc.vector.tensor_tensor(idxv[:], mask[:], cv("iota_desc"), op=OP.mult)
            rmax = small.tile([BL, 1], f32, tag="rmax")
            nc.vector.tensor_reduce(rmax[:], idxv[:], axis=AX.X, op=OP.max)
            nc.vector.tensor_scalar(ptrs_f[:, t:t + 1], rmax[:], -1.0, float(N - 1),
                                    op0=OP.mult, op1=OP.add)
            oh = small.tile([BL, N], f32, tag="oh")
            nc.vector.tensor_scalar(oh[:], cv("iota_rep"), ptrs_f[:, t:t + 1],
                                    None, op0=OP.is_equal)
            lprod = small.tile([BL, S * N], f32, tag="lprod")
            nc.vector.tensor_tensor(lprod[:], _sbcast(oh[:], S), static_bsn,
                                    op=OP.mult)
            last_b = small.tile([BL, S], f32, tag="last_b")
            nc.vector.tensor_reduce(last_b[:], lprod[:].rearrange("p (s n) -> p s n", s=S),
                                    axis=AX.X, op=OP.add)
            msc3 = mscp.tile([128, 512], f32, tag="msc")
            mm(msc3[:S, 128:128 + BL], last_b[:], ident, is_transpose=True)
            nc.vector.tensor_copy(lastT[:], msc3[:S, 128:128 + BL])

        ptr_i = state.tile([BL, N], i32)
        nc.vector.tensor_copy(ptr_i[:], ptrs_f[:])
        nc.sync.dma_start(out_idx[:], ptr_i[:])
        lg = state.tile([BL, N], f32)
        nc.scalar.activation(lg[:], s2s[:], AF.Ln)
        lsum = state.tile([BL, 1], f32)
        nc.vector.tensor_reduce(lsum[:], lg[:], axis=AX.X, op=OP.add)
        logp = state.tile([BL, 1], f32)
        nc.vector.tensor_scalar(logp[:], lsum[:], -1.0, None, op0=OP.mult)
        nc.sync.dma_start(out_logp[:], logp[:])

    nc.compile()
    return nc


def _host_prep(inputs):
    f = np.float32
    g = {k: np.asarray(v) for k, v in inputs.items()}
    attn_W, dec_W = g["attn_W"].astype(np.float64), g["dec_W"].astype(np.float64)
    enc_s_w, enc_d_w = g["enc_s_w"].astype(np.float64), g["enc_d_w"].astype(np.float64)
    W1, W2 = attn_W[:, :H], attn_W[:, H:2 * H]
    V1 = dec_W[:, :H]
    A1 = np.concatenate([W1 @ enc_s_w, W2 @ enc_d_w], axis=1).astype(f)
    A2 = (V1 @ enc_s_w).astype(f)
    bias1 = (W1 @ g["enc_s_b"].astype(np.float64)
             + W2 @ g["enc_d_b"].astype(np.float64)).astype(f)
    bias2 = (V1 @ g["enc_s_b"].astype(np.float64)).astype(f)

    A1_stack = np.zeros((128, H), f)
    A2_stack = np.zeros((128, H), f)
    for q in range(4):
        A1_stack[32 * q:32 * q + 3, :] = A1.T
        A2_stack[32 * q:32 * q + 2, :] = A2.T

    gbias = np.zeros((4 * H,), f)
    gbias[0:2 * H] = (g["b_ih"][0:2 * H] + g["b_hh"][0:2 * H]).astype(f)
    gbias[2 * H:3 * H] = g["b_ih"][2 * H:3 * H].astype(f)
    gbias[3 * H:4 * H] = g["b_hh"][2 * H:3 * H].astype(f)

    iota = np.arange(N, dtype=f)
    shared = {
        "A1_stack": A1_stack, "A2_stack": A2_stack,
        "ident": np.eye(H, dtype=f),
        "W3T": np.ascontiguousarray(g["attn_W"][:, 2 * H:].T.astype(f)),
        "V2T": np.ascontiguousarray(g["dec_W"][:, H:].T.astype(f)),
        "wihT": np.ascontiguousarray(g["w_ih"].T.astype(f)),
        "whhT": np.ascontiguousarray(g["w_hh"].T.astype(f)),
        "embT_w": np.ascontiguousarray(g["emb_w"].T.astype(f)),
        "encswT": np.ascontiguousarray(g["enc_s_w"].T.astype(f)),
        "v_col": np.ascontiguousarray(g["attn_v"].T.astype(f)),
        "dv_col": np.ascontiguousarray(g["dec_v"].T.astype(f)),
        "bias1_row": bias1[None, :], "bias2_row": bias2[None, :],
        "embb_row": g["emb_b"].astype(f)[None, :],
        "encsb_row": g["enc_s_b"].astype(f)[None, :],
        "ones_row": np.ones((1, BL), f),
        "gbias_rep": np.broadcast_to(gbias, (128, 4 * H)),
        "x0T_rep": np.broadcast_to(g["x0"].astype(f).T, (S, BL)),
        "iota_rep": np.broadcast_to(iota, (128, N)),
        "iota_desc": np.broadcast_to((N - 1) - iota, (128, N)),
    }

    static = g["static"].astype(f)
    dynamic = g["dynamic"].astype(f)
    in_maps = []
    for c in range(NCORES):
        sl = slice(c * BL, (c + 1) * BL)
        st, dy = static[sl], dynamic[sl]
        featb = np.stack([st[:, 0, :].reshape(-1), st[:, 1, :].reshape(-1),
                          dy[:, 0, :].reshape(-1)], axis=0)
        per = dict(shared)
        per["static_bsn"] = st.reshape(BL, S * N)
        blobarr = np.zeros((128, _BLOB_COLS), f)
        for nm, (c0, rows, w) in _SEG_OFF.items():
            blobarr[:rows, c0:c0 + w] = per[nm]
        in_maps.append({"blob": blobarr, "featd": featb})
    return in_maps


def kernel(**inputs):
    if "nc" not in _cache:
        _cache["nc"] = _build_program()
    in_maps = _host_prep(inputs)
    res = run_bass_kernel_spmd(_cache["nc"], in_maps, list(range(NCORES)))
    idx = np.concatenate([r["out_idx"] for r in res.results], axis=0)
    logp = np.concatenate([r["out_logp"][:, 0] for r in res.results], axis=0)
    return idx.astype(np.int32), logp.astype(np.float32)
